# revision 1
# baseline (speedup 1.0000x reference)
"""ChildSum TreeLSTM cell on 8 Trainium2 NeuronCores (Bass/Tile).

Sharding (hardcoded for N=100000, ENC=EMB=512, 8 cores):
  - Nodes are partitioned into 8 contiguous ranges; edges go to the core that
    owns their parent.  Weights are replicated.  h/c are replicated into each
    core's HBM so child gathers are local indirect DMAs (the all-to-all of
    the sharding hint is resolved by replication).  x is sharded by node
    range (leaves only ever read their own core's rows).
  - Per core, nodes are reordered: internal nodes (>=1 child) sorted by
    degree descending, then leaves.  The host does integer planning only
    (sorts, index/selector construction); all FP math runs on device.
  - Internal nodes are processed in groups of 128.  A group's edges are a
    contiguous run of the parent-sorted edge list, padded to a multiple of
    128.  The ragged per-parent reduction (segment sum) is a matmul with a
    0/1 selector S^T built on host.  Groups whose nodes all have exactly one
    child skip the selector matmul (h_tilde == gathered child row).
  - f = sigmoid(h_src @ U_f^T + b) needs feature-major h_src; [128,128]
    blocks are transposed on the PE array.  Matmuls run as float32r.
  - Outputs are written in permuted slot order and un-permuted on host.
"""

import sys

_TRN_REPO = "/opt/trn_rl_repo"
if _TRN_REPO not in sys.path:
    sys.path.insert(0, _TRN_REPO)

import numpy as np

P = 128
NCORES = 8
ENC = 512
KC = ENC // P  # 4 contraction chunks of 128

_LAST = {}  # debug/timing stash: nc + in_maps of the most recent kernel() call


# ------------------------------------------------------------- host planning


def _plan(ci, pi, n):
    npc = (n + NCORES - 1) // NCORES
    deg = np.bincount(pi, minlength=n)

    plans = []
    for c in range(NCORES):
        lo, hi = c * npc, min((c + 1) * npc, n)
        gids = np.arange(lo, hi, dtype=np.int64)
        d = deg[lo:hi]
        int_ids = gids[d > 0]
        int_ids = int_ids[np.argsort(-deg[int_ids], kind="stable")]
        leaf_ids = gids[d == 0]

        emask = (pi >= lo) & (pi < hi)
        e_child = ci[emask]
        e_parent = pi[emask]
        slot_of = np.full(hi - lo, -1, dtype=np.int64)
        slot_of[int_ids - lo] = np.arange(len(int_ids))
        e_slot = slot_of[e_parent - lo]
        eorder = np.argsort(e_slot, kind="stable")
        plans.append(
            {
                "lo": lo,
                "int_ids": int_ids,
                "leaf_ids": leaf_ids,
                "e_child": e_child[eorder],
                "e_slot": e_slot[eorder],
            }
        )

    g_int = max((len(p["int_ids"]) + P - 1) // P for p in plans)
    g_leaf = max(1, max((len(p["leaf_ids"]) + P - 1) // P for p in plans))

    # Cross-core uniform group metadata.
    chunks = np.ones(g_int, dtype=np.int64)
    deg1 = np.ones(g_int, dtype=bool)
    for p in plans:
        degs = deg[p["int_ids"]]
        starts = np.searchsorted(p["e_slot"], np.arange(g_int) * P)
        ends = np.searchsorted(p["e_slot"], (np.arange(g_int) + 1) * P)
        cnt = ends - starts
        chunks = np.maximum(chunks, (cnt + P - 1) // P)
        for g in range(g_int):
            sl = degs[g * P : (g + 1) * P]
            if len(sl) and not np.all(sl == 1):
                deg1[g] = False
        p["starts"], p["ends"] = starts, ends
    chunks[deg1] = 1

    eo = np.zeros(g_int + 1, dtype=np.int64)
    np.cumsum(chunks * P, out=eo[1:])
    nch = int(eo[-1]) // P  # total edge chunks

    so = np.full(g_int, -1, dtype=np.int64)
    s_rows = 0
    for g in np.flatnonzero(~deg1):
        so[g] = s_rows
        s_rows += int(chunks[g]) * P
    s_rows = max(s_rows, P)

    for p in plans:
        eidx = np.zeros(nch * P, dtype=np.int32)
        st = np.zeros((s_rows, P), dtype=np.float32)
        for g in range(g_int):
            s, e = int(p["starts"][g]), int(p["ends"][g])
            cnt = e - s
            eidx[eo[g] : eo[g] + cnt] = p["e_child"][s:e]
            if not deg1[g]:
                cols = p["e_slot"][s:e] - g * P
                st[so[g] + np.arange(cnt), cols] = 1.0
        # device layout: idx_all[p, j] = index of edge j*P + p
        p["eidx"] = np.ascontiguousarray(eidx.reshape(nch, P).T)
        p["st"] = st

    return plans, {
        "n": n,
        "npc": npc,
        "g_int": g_int,
        "g_leaf": g_leaf,
        "chunks": chunks,
        "deg1": deg1,
        "eo": eo,
        "so": so,
        "s_rows": s_rows,
        "nch": nch,
    }


# ---------------------------------------------------------- device program


def _build(meta, bias_iou_nonzero):
    from concourse import bass, bacc, tile, mybir
    from concourse.masks import make_identity

    f32 = mybir.dt.float32
    f32r = mybir.dt.float32r
    i32 = mybir.dt.int32
    AF = mybir.ActivationFunctionType

    n, npc = meta["n"], meta["npc"]
    g_int, g_leaf = meta["g_int"], meta["g_leaf"]
    chunks, deg1, eo, so = meta["chunks"], meta["deg1"], meta["eo"], meta["so"]
    nch = meta["nch"]
    slots = (g_int + g_leaf) * P

    nc = bacc.Bacc("TRN2", target_bir_lowering=False, debug=False)

    # float32r tensors hold plain fp32 bits; the PE rounds on read.  Every
    # SBUF tile the PE consumes must be produced by DMA, ACT, or DVE
    # tensor_copy (DVE tensor_tensor-produced f32r is PE-unreadable on HW).
    h_full = nc.dram_tensor("h_full", [n, ENC], f32r, kind="ExternalInput")
    c_full = nc.dram_tensor("c_full", [n, ENC], f32, kind="ExternalInput")
    xt_d = nc.dram_tensor("xt", [g_leaf * ENC, P], f32r, kind="ExternalInput")
    eidx_d = nc.dram_tensor("eidx", [P, nch], i32, kind="ExternalInput")
    st_d = nc.dram_tensor("st", [meta["s_rows"], P], f32r, kind="ExternalInput")
    wf_d = nc.dram_tensor("wfT", [ENC, ENC], f32r, kind="ExternalInput")
    wi_d = nc.dram_tensor("wiT", [ENC, 3 * ENC], f32r, kind="ExternalInput")
    wx_d = nc.dram_tensor("wxT", [ENC, 3 * ENC], f32r, kind="ExternalInput")
    bf_d = nc.dram_tensor("bf", [1, ENC], f32r, kind="ExternalInput")
    bi_d = nc.dram_tensor("bi", [1, 3 * ENC], f32r, kind="ExternalInput")
    yh_d = nc.dram_tensor("yh", [slots, ENC], f32, kind="ExternalOutput")
    yc_d = nc.dram_tensor("yc", [slots, ENC], f32, kind="ExternalOutput")

    with tile.TileContext(nc) as tc:
        with (
            tc.tile_pool(name="const", bufs=1) as cp,
            tc.tile_pool(name="work", bufs=3) as wp,
            tc.tile_pool(name="pt", bufs=2, space="PSUM") as pt_p,
            tc.tile_pool(name="pf", bufs=1, space="PSUM") as pf_p,
            tc.tile_pool(name="pseg", bufs=1, space="PSUM") as pseg_p,
            tc.tile_pool(name="piou", bufs=1, space="PSUM") as piou_p,
        ):
            ident_f = cp.tile([P, P], f32, name="ident_f")
            make_identity(nc, ident_f[:])
            ident = cp.tile([P, P], f32r, name="ident")
            nc.vector.tensor_copy(out=ident[:], in_=ident_f[:])
            ones_f = cp.tile([1, P], f32, name="ones_f")
            nc.gpsimd.memset(ones_f[:], 1.0)
            ones_row = cp.tile([1, P], f32r, name="ones_row")
            nc.vector.tensor_copy(out=ones_row[:], in_=ones_f[:])
            idx_all = cp.tile([P, nch], i32, name="idx_all")
            nc.sync.dma_start(out=idx_all[:], in_=eidx_d[:])
            wf, wi, wx = [], [], []
            for k in range(KC):
                t = cp.tile([P, ENC], f32r, tag=f"wf{k}", name=f"wf{k}")
                nc.sync.dma_start(out=t[:], in_=wf_d[k * P : (k + 1) * P, :])
                wf.append(t)
                t = cp.tile([P, 3 * ENC], f32r, tag=f"wi{k}", name=f"wi{k}")
                nc.sync.dma_start(out=t[:], in_=wi_d[k * P : (k + 1) * P, :])
                wi.append(t)
                t = cp.tile([P, 3 * ENC], f32r, tag=f"wx{k}", name=f"wx{k}")
                nc.sync.dma_start(out=t[:], in_=wx_d[k * P : (k + 1) * P, :])
                wx.append(t)
            bf_t = cp.tile([1, ENC], f32r, name="bf_t")
            nc.sync.dma_start(out=bf_t[:], in_=bf_d[:])
            bi_t = cp.tile([1, 3 * ENC], f32r, name="bi_t")
            nc.sync.dma_start(out=bi_t[:], in_=bi_d[:])

            def transpose_to(dst, dst_col, src, src_col):
                """dst[:, dst_col:+P] = src[:, src_col:+P].T (PE, f32r)."""
                pt = pt_p.tile([P, P], f32r, space="PSUM", tag="pt", name="pt")
                nc.tensor.transpose(
                    out=pt[:], in_=src[:, src_col : src_col + P], identity=ident[:]
                )
                nc.vector.tensor_copy(out=dst[:, dst_col : dst_col + P], in_=pt[:])

            def iou_mms(piou, lhs_cols, w_tiles):
                """piou[b] (+)= sum_k lhs_cols[k].T @ w_tiles[k][:, b*ENC:+ENC]."""
                for k in range(KC):
                    last = (k == KC - 1) and not bias_iou_nonzero
                    for b in range(3):
                        nc.tensor.matmul(
                            out=piou[b][:],
                            lhsT=lhs_cols[k],
                            rhs=w_tiles[k][:, b * ENC : (b + 1) * ENC],
                            start=(k == 0),
                            stop=last,
                        )
                if bias_iou_nonzero:
                    for b in range(3):
                        nc.tensor.matmul(
                            out=piou[b][:],
                            lhsT=ones_row[:],
                            rhs=bi_t[:, b * ENC : (b + 1) * ENC],
                            start=False,
                            stop=True,
                        )

            def iou_tail(piou, c_red, row0):
                """c_new = sig(i)*tanh(u) (+ c_red); h_new = sig(o)*tanh(c_new)."""
                si = wp.tile([P, ENC], f32, tag="si", name="si")
                nc.scalar.activation(si[:], piou[0][:], AF.Sigmoid)
                so_t = wp.tile([P, ENC], f32, tag="so", name="so")
                nc.scalar.activation(so_t[:], piou[1][:], AF.Sigmoid)
                tu = wp.tile([P, ENC], f32, tag="tu", name="tu")
                nc.scalar.activation(tu[:], piou[2][:], AF.Tanh)
                cn = wp.tile([P, ENC], f32, tag="cn", name="cn")
                nc.vector.tensor_mul(out=cn[:], in0=si[:], in1=tu[:])
                if c_red is not None:
                    nc.vector.tensor_add(out=cn[:], in0=cn[:], in1=c_red[:])
                tc_t = wp.tile([P, ENC], f32, tag="tc", name="tc")
                nc.scalar.activation(tc_t[:], cn[:], AF.Tanh)
                hn = wp.tile([P, ENC], f32, tag="hn", name="hn")
                nc.vector.tensor_mul(out=hn[:], in0=so_t[:], in1=tc_t[:])
                nc.sync.dma_start(out=yc_d[row0 : row0 + P, :], in_=cn[:])
                nc.sync.dma_start(out=yh_d[row0 : row0 + P, :], in_=hn[:])

            # ------------- internal groups -------------
            def emit_internal(g):
                ck = int(chunks[g])
                j0 = int(eo[g]) // P
                is_d1 = bool(deg1[g])

                hs = []
                for ec in range(ck):
                    t = wp.tile([P, ENC], f32r, tag=f"hs{ec}", name=f"hs{ec}")
                    nc.gpsimd.indirect_dma_start(
                        out=t[:],
                        out_offset=None,
                        in_=h_full[:],
                        in_offset=bass.IndirectOffsetOnAxis(
                            ap=idx_all[:, j0 + ec : j0 + ec + 1], axis=0
                        ),
                    )
                    hs.append(t)

                if not is_d1:
                    st_g = wp.tile([P, ck * P], f32r, tag="st_g", name="st_g")
                    for ec in range(ck):
                        nc.sync.dma_start(
                            out=st_g[:, ec * P : (ec + 1) * P],
                            in_=st_d[
                                int(so[g]) + ec * P : int(so[g]) + (ec + 1) * P, :
                            ],
                        )
                    pcr = pseg_p.tile(
                        [P, ENC], f32, space="PSUM", tag="pcr", name="pcr"
                    )

                # feature-major h_src for the f matmul (and iou lhsT when deg1)
                hT = [
                    wp.tile([P, ck * P], f32r, tag=f"hT{k}", name=f"hT{k}")
                    for k in range(KC)
                ]
                fcm_d1 = None
                for ec in range(ck):
                    for k in range(KC):
                        transpose_to(hT[k], ec * P, hs[ec], k * P)
                    cs = wp.tile([P, ENC], f32, tag="cs", name="cs", bufs=3)
                    nc.gpsimd.indirect_dma_start(
                        out=cs[:],
                        out_offset=None,
                        in_=c_full[:],
                        in_offset=bass.IndirectOffsetOnAxis(
                            ap=idx_all[:, j0 + ec : j0 + ec + 1], axis=0
                        ),
                    )
                    pf = pf_p.tile([P, ENC], f32, space="PSUM", tag="pf", name="pf")
                    for k in range(KC):
                        nc.tensor.matmul(
                            out=pf[:],
                            lhsT=hT[k][:, ec * P : (ec + 1) * P],
                            rhs=wf[k][:],
                            start=(k == 0),
                            stop=False,
                        )
                    nc.tensor.matmul(
                        out=pf[:],
                        lhsT=ones_row[:],
                        rhs=bf_t[:],
                        start=False,
                        stop=True,
                    )
                    f_t = wp.tile([P, ENC], f32, tag="f_t", name="f_t")
                    nc.scalar.activation(f_t[:], pf[:], AF.Sigmoid)
                    fcm = wp.tile([P, ENC], f32, tag="fcm", name="fcm", bufs=3)
                    nc.vector.tensor_mul(out=fcm[:], in0=f_t[:], in1=cs[:])
                    if is_d1:
                        fcm_d1 = fcm
                    else:
                        fcr = wp.tile(
                            [P, ENC], f32r, tag="fcr", name="fcr", bufs=3
                        )
                        nc.scalar.copy(out=fcr[:], in_=fcm[:])
                        nc.tensor.matmul(
                            out=pcr[:],
                            lhsT=st_g[:, ec * P : (ec + 1) * P],
                            rhs=fcr[:],
                            start=(ec == 0),
                            stop=(ec == ck - 1),
                        )

                piou = [
                    piou_p.tile(
                        [P, ENC], f32, space="PSUM", tag=f"piou{b}", name=f"piou{b}"
                    )
                    for b in range(3)
                ]
                if is_d1:
                    # every node has exactly one child: h_tilde==h_src, c_red==fc
                    iou_mms(piou, [hT[k][:, 0:P] for k in range(KC)], wi)
                    iou_tail(piou, fcm_d1[:], g * P)
                else:
                    # h_tilde, transposed directly: phtT[:, k*P:+P] = sum_e
                    # hs[e, kP:+P]^T S[e, :]  (k-outer so same-bank accumulation
                    # groups do not interleave their start bits)
                    phtT = pseg_p.tile(
                        [P, ENC], f32, space="PSUM", tag="phtT", name="phtT"
                    )
                    for k in range(KC):
                        for ec in range(ck):
                            nc.tensor.matmul(
                                out=phtT[:, k * P : (k + 1) * P],
                                lhsT=hs[ec][:, k * P : (k + 1) * P],
                                rhs=st_g[:, ec * P : (ec + 1) * P],
                                start=(ec == 0),
                                stop=(ec == ck - 1),
                            )
                    htT = wp.tile([P, KC * P], f32r, tag="htT", name="htT")
                    nc.scalar.copy(out=htT[:], in_=phtT[:])
                    iou_mms(
                        piou, [htT[:, k * P : (k + 1) * P] for k in range(KC)], wi
                    )
                    iou_tail(piou, pcr[:], g * P)

            # ------------- leaf groups -------------
            def emit_leaf(g):
                xT = wp.tile([P, KC * P], f32r, tag="xT", name="xT")
                for k in range(KC):
                    nc.sync.dma_start(
                        out=xT[:, k * P : (k + 1) * P],
                        in_=xt_d[g * ENC + k * P : g * ENC + (k + 1) * P, :],
                    )
                piou = [
                    piou_p.tile(
                        [P, ENC], f32, space="PSUM", tag=f"piou{b}", name=f"piou{b}"
                    )
                    for b in range(3)
                ]
                iou_mms(piou, [xT[:, k * P : (k + 1) * P] for k in range(KC)], wx)
                iou_tail(piou, None, (g_int + g) * P)

            # Interleave leaf groups (no gather dependencies) among internal
            # groups so the PE has independent matmul work during internal
            # groups' gather/tail stalls.  Emission order only; numerics
            # identical.
            li = 0
            for g in range(g_int):
                emit_internal(g)
                while li * g_int < (g + 1) * g_leaf and li < g_leaf:
                    emit_leaf(li)
                    li += 1
            while li < g_leaf:
                emit_leaf(li)
                li += 1

    nc.compile()
    return nc


# ------------------------------------------------------------------ kernel


def kernel(x, h, c, child_idx, parent_idx, W_iou, U_iou, b_iou, U_f_w, U_f_b):
    from concourse.bass_utils import run_bass_kernel_spmd

    x = np.ascontiguousarray(np.asarray(x, dtype=np.float32))
    h = np.ascontiguousarray(np.asarray(h, dtype=np.float32))
    c = np.ascontiguousarray(np.asarray(c, dtype=np.float32))
    ci = np.asarray(child_idx, dtype=np.int64)
    pi = np.asarray(parent_idx, dtype=np.int64)
    W_iou = np.asarray(W_iou, dtype=np.float32)
    U_iou = np.asarray(U_iou, dtype=np.float32)
    b_iou = np.asarray(b_iou, dtype=np.float32)
    U_f_w = np.asarray(U_f_w, dtype=np.float32)
    U_f_b = np.asarray(U_f_b, dtype=np.float32)

    n = x.shape[0]
    plans, meta = _plan(ci, pi, n)
    npc = meta["npc"]
    nc = _build(meta, bool(np.any(b_iou != 0.0)))

    wfT = np.ascontiguousarray(U_f_w.T)
    wiT = np.ascontiguousarray(U_iou.T)
    wxT = np.ascontiguousarray(W_iou.T)
    bf = np.ascontiguousarray(U_f_b.reshape(1, ENC))
    bi = np.ascontiguousarray(b_iou.reshape(1, 3 * ENC))

    g_leaf = meta["g_leaf"]
    in_maps = []
    for p in plans:
        # pre-transposed leaf features in slot order: xt[g*ENC+f, s] = x[leaf_s, f]
        xt = np.zeros((g_leaf * ENC, P), dtype=np.float32)
        nl = len(p["leaf_ids"])
        xg = np.zeros((g_leaf * P, ENC), dtype=np.float32)
        xg[:nl] = x[p["leaf_ids"]]
        for g in range(g_leaf):
            xt[g * ENC : (g + 1) * ENC, :] = xg[g * P : (g + 1) * P].T
        in_maps.append(
            {
                "h_full": h,
                "c_full": c,
                "xt": xt,
                "eidx": p["eidx"],
                "st": p["st"],
                "wfT": wfT,
                "wiT": wiT,
                "wxT": wxT,
                "bf": bf,
                "bi": bi,
            }
        )

    _LAST.update(nc=nc, in_maps=in_maps, plans=plans, meta=meta)
    res = run_bass_kernel_spmd(nc, in_maps, core_ids=list(range(NCORES)))

    H = np.empty((n, ENC), dtype=np.float32)
    C = np.empty((n, ENC), dtype=np.float32)
    g_int = meta["g_int"]
    for p, out in zip(plans, res.results):
        yh, yc = out["yh"], out["yc"]
        ni, nl = len(p["int_ids"]), len(p["leaf_ids"])
        H[p["int_ids"]] = yh[:ni]
        C[p["int_ids"]] = yc[:ni]
        H[p["leaf_ids"]] = yh[g_int * P : g_int * P + nl]
        C[p["leaf_ids"]] = yc[g_int * P : g_int * P + nl]
    return H, C

